# revision 2
# baseline (speedup 1.0000x reference)
"""Trainium2 Bass kernel for nn_Attention_81750407512209.

Full attention: out = softmax((x Wq)(x Wk)^T / sqrt(128)) @ (x Wv)
B=8 batches sharded 1:1 onto 8 NeuronCores (data parallel, weights replicated).

Per-core design (N=4096 ctx, D=128):
  - x^T via PE transpose; Q^T/K^T projections in float32r (full PE speed at
    out free >=256, ~1.5e-4 matmul rel err measured on silicon); 1/sqrt(128)
    folded into Wq.
  - Scores per 128-row q-tile in four [128,1024] PSUM chunks (2 banks each,
    pool bufs=3 -> 6 banks).  Row max via DVE reduce_max(negate=True).
  - "Flash-lite" softmax: chunks 0,1 exponentiated with bias -max(c0,c1),
    chunks 2,3 with the full -max; one PSUM rescale by
    gamma = exp(max01 - max) between the two AV accumulation halves.
  - P = exp(S + bias) on ScalarE, PSUM -> SBUF bf16.
  - P^T via wide xbar DMA transposes ([128,2048] -> [128,16,128] batched
    block transpose) on the sync HWDGE engine (xbar is a serialized
    ~115GB/s resource; never dual-issue - corrupts), optionally a tail of
    kv-tiles via PE transposes + PSUM->SBUF copies (hybrid, PE_KV_TILES).
  - AV: 32 bf16 matmuls lhsT=P^T tile [kv,q], rhs=V tile augmented with a
    ones column -> row sums accumulate in PSUM col 128.  Normalize with
    DVE reciprocal + ScalarE copy*scale.
  - Software pipelined: q-tile i-1's AV/normalize emitted after q-tile i's
    score work so PE is never blocked on the softmax chain.
"""

import numpy as np
from contextlib import ExitStack

import concourse.bass as bass
import concourse.tile as tile
from concourse import bacc, mybir
from concourse.bass_utils import run_bass_kernel_spmd
from concourse.masks import make_identity

F32 = mybir.dt.float32
F32R = mybir.dt.float32r
BF16 = mybir.dt.bfloat16
AX = mybir.AxisListType.X
OP = mybir.AluOpType
AF = mybir.ActivationFunctionType

B, N, D = 8, 4096, 128
NT = N // 128          # 32 kv/q tiles
CH = 1024              # score chunk (2 PSUM banks)
# kv-tiles whose P^T is produced by PE transpose + copies instead of the xbar
# (taken from the END of the kv range; must be multiple of 4)
PE_KV_TILES = 0
SCALE = 1.0 / np.sqrt(np.float32(D))


def build_attention(nc: bacc.Bacc, pe_kv_tiles=PE_KV_TILES):
    x = nc.dram_tensor("x", [N, D], F32, kind="ExternalInput").ap()
    wq = nc.dram_tensor("w_query", [D, D], F32, kind="ExternalInput").ap()
    wk = nc.dram_tensor("w_key", [D, D], F32, kind="ExternalInput").ap()
    wv = nc.dram_tensor("w_value", [D, D], F32, kind="ExternalInput").ap()
    out = nc.dram_tensor("out", [N, D], F32, kind="ExternalOutput").ap()

    xb_tiles = NT - pe_kv_tiles  # kv-tiles transposed by the xbar
    assert pe_kv_tiles % 4 == 0 and xb_tiles % 4 == 0

    with tile.TileContext(nc) as tc, ExitStack() as ctx:
        consts = ctx.enter_context(tc.tile_pool(name="consts", bufs=1))
        big = ctx.enter_context(tc.tile_pool(name="big", bufs=1))
        xin = ctx.enter_context(tc.tile_pool(name="xin", bufs=4))
        pbuf = ctx.enter_context(tc.tile_pool(name="pbuf", bufs=2))
        stats = ctx.enter_context(tc.tile_pool(name="stats", bufs=3))
        ostage = ctx.enter_context(tc.tile_pool(name="ostage", bufs=3))

        ident = consts.tile([128, 128], F32, name="ident")
        make_identity(nc, ident[:])
        identb = consts.tile([128, 128], BF16, name="identb")
        nc.vector.tensor_copy(identb[:], ident[:])

        wq_st = consts.tile([128, 128], F32, name="wq_st")
        wk_st = consts.tile([128, 128], F32, name="wk_st")
        wv_st = consts.tile([128, 128], F32, name="wv_st")
        nc.sync.dma_start(wq_st[:], wq[:])
        nc.sync.dma_start(wk_st[:], wk[:])
        nc.sync.dma_start(wv_st[:], wv[:])
        wq_r = consts.tile([128, 128], F32R, name="wq_r")
        wk_r = consts.tile([128, 128], F32R, name="wk_r")
        wv_r = consts.tile([128, 128], F32R, name="wv_r")
        nc.vector.tensor_scalar_mul(wq_r[:], wq_st[:], float(SCALE))
        nc.vector.tensor_copy(wk_r[:], wk_st[:])
        nc.vector.tensor_copy(wv_r[:], wv_st[:])

        xT = big.tile([128, N], F32R, name="xT")
        kT = big.tile([128, N], F32R, name="kT")
        qT = big.tile([128, N], F32R, name="qT")
        vaug = big.tile([128, NT, 129], BF16, name="vaug")
        nc.gpsimd.memset(vaug[:, :, 128:129], 1.0)

        # ---- prologue: x^T, projections (scoped PSUM pool) ----
        with tc.tile_pool(name="ps_pro", bufs=2, space="PSUM") as ps_pro:
            for i in range(NT):
                xt = xin.tile([128, 128], F32, tag="xt", name="xt")
                nc.sync.dma_start(xt[:], x[i * 128:(i + 1) * 128, :])
                ps = ps_pro.tile([128, 128], F32, tag="xtp", name="xtp")
                nc.tensor.transpose(ps[:], xt[:], ident[:])
                nc.vector.tensor_copy(xT[:, i * 128:(i + 1) * 128], ps[:])
            for c in range(N // 512):
                sl = slice(c * 512, (c + 1) * 512)
                pk = ps_pro.tile([128, 512], F32, tag="proj", name="pk")
                nc.tensor.matmul(pk[:], wk_r[:], xT[:, sl], start=True, stop=True)
                nc.vector.tensor_copy(kT[:, sl], pk[:])
            for c in range(N // 512):
                sl = slice(c * 512, (c + 1) * 512)
                pq = ps_pro.tile([128, 512], F32, tag="proj", name="pq")
                nc.tensor.matmul(pq[:], wq_r[:], xT[:, sl], start=True, stop=True)
                nc.vector.tensor_copy(qT[:, sl], pq[:])
            for i in range(NT):
                pv = ps_pro.tile([128, 128], F32, tag="vproj", name="pv")
                nc.tensor.matmul(
                    pv[:], xT[:, i * 128:(i + 1) * 128], wv_r[:],
                    start=True, stop=True,
                )
                nc.scalar.copy(vaug[:, i, 0:128], pv[:])

        # ---- main loop pools ----
        ps_s = ctx.enter_context(tc.tile_pool(name="ps_s", bufs=3, space="PSUM"))
        ps_av = ctx.enter_context(tc.tile_pool(name="ps_av", bufs=2, space="PSUM"))
        if pe_kv_tiles:
            ps_pt = ctx.enter_context(tc.tile_pool(name="ps_pt", bufs=2, space="PSUM"))

        def score_chunk(qsl, c):
            s = ps_s.tile([128, CH], F32, tag="sh", name="sh")
            for k in range(CH // 512):
                nc.tensor.matmul(
                    s[:, k * 512:(k + 1) * 512],
                    qsl,
                    kT[:, c * CH + k * 512: c * CH + (k + 1) * 512],
                    start=True,
                    stop=True,
                )
            return s

        def negmax(s, tg):
            nm = stats.tile([128, 1], F32, tag=tg, name="nm")
            nc.vector.reduce_max(nm[:], s[:], axis=AX, negate=True)
            return nm

        pend = None
        for i in range(NT + 1):
            if i < NT:
                qsl = qT[:, i * 128:(i + 1) * 128]
                P = pbuf.tile([128, N], BF16, tag="P", name="P")
                PT = pbuf.tile([128, NT, 128], BF16, tag="PT", name="PT")

                s0 = score_chunk(qsl, 0)
                s1 = score_chunk(qsl, 1)
                n0 = negmax(s0, "n0")
                n1 = negmax(s1, "n1")
                b01 = stats.tile([128, 1], F32, tag="b01", name="b01")
                nc.vector.tensor_tensor(b01[:], n0[:], n1[:], op=OP.min)
                nc.scalar.activation(P[:, 0:CH], s0[:], AF.Exp, bias=b01[:])
                nc.scalar.activation(P[:, CH:2 * CH], s1[:], AF.Exp, bias=b01[:])
                # first xbar transpose as soon as the first half of P exists
                nc.sync.dma_start_transpose(PT[:, 0:16, :], P[:, 0:2048])

                s2 = score_chunk(qsl, 2)
                s3 = score_chunk(qsl, 3)
                n2 = negmax(s2, "n2")
                n3 = negmax(s3, "n3")
                b23 = stats.tile([128, 1], F32, tag="b23", name="b23")
                nc.vector.tensor_tensor(b23[:], n2[:], n3[:], op=OP.min)
                bias = stats.tile([128, 1], F32, tag="bias", name="bias")
                nc.vector.tensor_tensor(bias[:], b01[:], b23[:], op=OP.min)
                gin = stats.tile([128, 1], F32, tag="gin", name="gin")
                nc.vector.tensor_tensor(gin[:], bias[:], b01[:], op=OP.subtract)
                gam = stats.tile([128, 1], F32, tag="gam", name="gam")
                nc.scalar.activation(gam[:], gin[:], AF.Exp)
                nc.scalar.activation(P[:, 2 * CH:3 * CH], s2[:], AF.Exp, bias=bias[:])
                nc.scalar.activation(P[:, 3 * CH:4 * CH], s3[:], AF.Exp, bias=bias[:])
                if xb_tiles > 16:
                    nc.sync.dma_start_transpose(
                        PT[:, 16:xb_tiles, :], P[:, 2048:xb_tiles * 128]
                    )
                # PE-transpose tail (hybrid): groups of 4 kv-tiles
                for g in range(pe_kv_tiles // 4):
                    t0 = xb_tiles + g * 4
                    pt_ps = ps_pt.tile([128, 512], BF16, tag="ptp", name="ptp")
                    for u in range(4):
                        nc.tensor.transpose(
                            pt_ps[:, u * 128:(u + 1) * 128],
                            P[:, (t0 + u) * 128:(t0 + u + 1) * 128],
                            identb[:],
                        )
                    eng = nc.vector if g % 2 == 0 else nc.scalar
                    eng.tensor_copy(
                        PT[:, t0:t0 + 4, :].rearrange("p a b -> p (a b)"), pt_ps[:]
                    )
                cur = (PT, gam, i)
            else:
                cur = None

            if pend is not None:
                PTp, gamp, j = pend
                av = ps_av.tile([128, 129], F32, tag="av", name="av")
                for t in range(16):
                    nc.tensor.matmul(
                        av[:], PTp[:, t, :], vaug[:, t, :],
                        start=(t == 0), stop=False,
                    )
                nc.scalar.activation(av[:], av[:], AF.Copy, bias=0.0, scale=gamp[:])
                for t in range(16, 32):
                    nc.tensor.matmul(
                        av[:], PTp[:, t, :], vaug[:, t, :],
                        start=False, stop=(t == 31),
                    )
                linv = stats.tile([128, 1], F32, tag="linv", name="linv")
                nc.vector.reciprocal(linv[:], av[:, 128:129])
                ost = ostage.tile([128, 128], F32, tag="ost", name="ost")
                nc.scalar.activation(
                    ost[:], av[:, 0:128], AF.Copy, bias=0.0, scale=linv[:]
                )
                nc.sync.dma_start(out[j * 128:(j + 1) * 128, :], ost[:])
            pend = cur

    nc.compile()
    return nc


_NC_CACHE = {}


def _get_nc():
    if "nc" not in _NC_CACHE:
        nc = bacc.Bacc("TRN2", target_bir_lowering=False, debug=False, num_devices=B)
        _NC_CACHE["nc"] = build_attention(nc)
    return _NC_CACHE["nc"]


def kernel(x, w_query, w_key, w_value, _trace=False):
    x = np.ascontiguousarray(np.asarray(x, dtype=np.float32))
    w_query = np.ascontiguousarray(np.asarray(w_query, dtype=np.float32))
    w_key = np.ascontiguousarray(np.asarray(w_key, dtype=np.float32))
    w_value = np.ascontiguousarray(np.asarray(w_value, dtype=np.float32))
    nc = _get_nc()
    in_maps = [
        {"x": x[b], "w_query": w_query, "w_key": w_key, "w_value": w_value}
        for b in range(B)
    ]
    res = run_bass_kernel_spmd(nc, in_maps, core_ids=list(range(B)), trace=_trace)
    out_full = np.stack([res.results[b]["out"] for b in range(B)])
    if _trace:
        kernel.last_exec_time_ns = res.exec_time_ns
    return out_full


# revision 3
# speedup vs baseline: 1.2818x; 1.2818x over previous
"""Trainium2 Bass kernel for nn_Attention_81750407512209.

Full attention: out = softmax((x Wq)(x Wk)^T / sqrt(128)) @ (x Wv)
B=8 batches sharded 1:1 onto 8 NeuronCores (data parallel, weights replicated).

Per-core design (N=4096 ctx, D=128):
  - x^T via PE transpose; Q^T/K^T projections computed in float32r
    (~1.5e-4 matmul rel err measured on silicon) then stored bf16;
    1/sqrt(128) folded into Wq.  Scores matmul runs bf16 (2-byte moving
    operand streams at 1 cyc/row vs ~2.4 for 4-byte) - measured end-to-end
    rel err ~2e-3 vs the f32 reference.
  - Scores per 128-row q-tile in PSUM chunks (1536,1536,1024) - pool of
    two 3-bank slots + the 1024 chunk reuses a freed slot.
  - Row max via DVE reduce_max(negate=True) per chunk.
  - "Flash-lite" softmax: chunks 0,1 exponentiated with bias -max(c0,c1),
    chunk 2 with the full row -max; single PSUM rescale of the AV
    accumulator by gamma = exp(max01 - max) between AV kv-halves.
  - P = exp(S + bias) on ScalarE, PSUM -> SBUF bf16.
  - P^T via wide xbar DMA transposes ([128,2048] -> [128,16,128] batched
    block transpose) on the sync HWDGE engine only (xbar is a serialized
    resource; dual-engine issue corrupts data - measured).
  - AV: 32 bf16 matmuls lhsT=P^T tile [kv,q], rhs=V tile augmented with a
    ones column -> row sums accumulate in PSUM col 128.  Normalize with
    DVE reciprocal + ScalarE copy*scale.
  - Software pipelined: q-tile i-1's AV/normalize emitted interleaved with
    q-tile i's score work so PE is never blocked on the softmax chain.
"""

import numpy as np
from contextlib import ExitStack

import concourse.bass as bass
import concourse.tile as tile
from concourse import bacc, mybir
from concourse.bass_utils import run_bass_kernel_spmd
from concourse.masks import make_identity

F32 = mybir.dt.float32
F32R = mybir.dt.float32r
BF16 = mybir.dt.bfloat16
AX = mybir.AxisListType.X
OP = mybir.AluOpType
AF = mybir.ActivationFunctionType

B, N, D = 8, 4096, 128
NT = N // 128                    # 32 kv/q tiles
CHUNKS = (1536, 1536, 1024)      # score chunks; first two share bias m01
SCALE = 1.0 / np.sqrt(np.float32(D))
RESCALE_T = (CHUNKS[0] + CHUNKS[1]) // 128   # kv-tile where gamma applies (24)


def build_attention(nc: bacc.Bacc):
    x = nc.dram_tensor("x", [N, D], F32, kind="ExternalInput").ap()
    wq = nc.dram_tensor("w_query", [D, D], F32, kind="ExternalInput").ap()
    wk = nc.dram_tensor("w_key", [D, D], F32, kind="ExternalInput").ap()
    wv = nc.dram_tensor("w_value", [D, D], F32, kind="ExternalInput").ap()
    out = nc.dram_tensor("out", [N, D], F32, kind="ExternalOutput").ap()

    with tile.TileContext(nc) as tc, ExitStack() as ctx:
        consts = ctx.enter_context(tc.tile_pool(name="consts", bufs=1))
        big = ctx.enter_context(tc.tile_pool(name="big", bufs=1))
        xin = ctx.enter_context(tc.tile_pool(name="xin", bufs=4))
        pbuf = ctx.enter_context(tc.tile_pool(name="pbuf", bufs=2))
        stats = ctx.enter_context(tc.tile_pool(name="stats", bufs=3))
        ostage = ctx.enter_context(tc.tile_pool(name="ostage", bufs=3))

        ident = consts.tile([128, 128], F32, name="ident")
        make_identity(nc, ident[:])

        wq_st = consts.tile([128, 128], F32, name="wq_st")
        wk_st = consts.tile([128, 128], F32, name="wk_st")
        wv_st = consts.tile([128, 128], F32, name="wv_st")
        nc.sync.dma_start(wq_st[:], wq[:])
        nc.sync.dma_start(wk_st[:], wk[:])
        nc.sync.dma_start(wv_st[:], wv[:])
        wq_r = consts.tile([128, 128], F32R, name="wq_r")
        wk_r = consts.tile([128, 128], F32R, name="wk_r")
        wv_r = consts.tile([128, 128], F32R, name="wv_r")
        nc.vector.tensor_scalar_mul(wq_r[:], wq_st[:], float(SCALE))
        nc.vector.tensor_copy(wk_r[:], wk_st[:])
        nc.vector.tensor_copy(wv_r[:], wv_st[:])

        xT = big.tile([128, N], F32R, name="xT")
        kT = big.tile([128, N], BF16, name="kT")
        qT = big.tile([128, N], BF16, name="qT")
        vaug = big.tile([128, NT, 129], BF16, name="vaug")
        nc.gpsimd.memset(vaug[:, :, 128:129], 1.0)

        # ---- prologue: x^T, projections (scoped PSUM pool) ----
        with tc.tile_pool(name="ps_pro", bufs=2, space="PSUM") as ps_pro:
            for i in range(NT):
                xt = xin.tile([128, 128], F32, tag="xt", name="xt")
                nc.sync.dma_start(xt[:], x[i * 128:(i + 1) * 128, :])
                ps = ps_pro.tile([128, 128], F32, tag="xtp", name="xtp")
                nc.tensor.transpose(ps[:], xt[:], ident[:])
                nc.vector.tensor_copy(xT[:, i * 128:(i + 1) * 128], ps[:])
            for c in range(N // 512):
                sl = slice(c * 512, (c + 1) * 512)
                pk = ps_pro.tile([128, 512], F32, tag="proj", name="pk")
                nc.tensor.matmul(pk[:], wk_r[:], xT[:, sl], start=True, stop=True)
                nc.vector.tensor_copy(kT[:, sl], pk[:])
            for c in range(N // 512):
                sl = slice(c * 512, (c + 1) * 512)
                pq = ps_pro.tile([128, 512], F32, tag="proj", name="pq")
                nc.tensor.matmul(pq[:], wq_r[:], xT[:, sl], start=True, stop=True)
                nc.vector.tensor_copy(qT[:, sl], pq[:])
            for i in range(NT):
                pv = ps_pro.tile([128, 128], F32, tag="vproj", name="pv")
                nc.tensor.matmul(
                    pv[:], xT[:, i * 128:(i + 1) * 128], wv_r[:],
                    start=True, stop=True,
                )
                nc.scalar.copy(vaug[:, i, 0:128], pv[:])

        # ---- main loop pools: 2x3-bank score slots + 2x1-bank AV accum ----
        ps_s = ctx.enter_context(tc.tile_pool(name="ps_s", bufs=2, space="PSUM"))
        ps_av = ctx.enter_context(tc.tile_pool(name="ps_av", bufs=2, space="PSUM"))

        def score_chunk(qsl, off, width):
            s = ps_s.tile([128, CHUNKS[0]], F32, tag="sh", name="sh")
            for k in range(width // 512):
                nc.tensor.matmul(
                    s[:, k * 512:(k + 1) * 512],
                    qsl,
                    kT[:, off + k * 512: off + (k + 1) * 512],
                    start=True,
                    stop=True,
                )
            return s

        def negmax(s, width, tg):
            nm = stats.tile([128, 1], F32, tag=tg, name="nm")
            nc.vector.reduce_max(nm[:], s[:, 0:width], axis=AX, negate=True)
            return nm

        pend = None
        for i in range(NT + 1):
            if i < NT:
                qsl = qT[:, i * 128:(i + 1) * 128]
                P = pbuf.tile([128, N], BF16, tag="P", name="P")
                PT = pbuf.tile([128, NT, 128], BF16, tag="PT", name="PT")

                s0 = score_chunk(qsl, 0, CHUNKS[0])
                s1 = score_chunk(qsl, CHUNKS[0], CHUNKS[1])
                n0 = negmax(s0, CHUNKS[0], "n0")
                n1 = negmax(s1, CHUNKS[1], "n1")
                b01 = stats.tile([128, 1], F32, tag="b01", name="b01")
                nc.vector.tensor_tensor(b01[:], n0[:], n1[:], op=OP.min)
                nc.scalar.activation(P[:, 0:CHUNKS[0]], s0[:], AF.Exp, bias=b01[:])
                nc.scalar.activation(
                    P[:, CHUNKS[0]:CHUNKS[0] + CHUNKS[1]],
                    s1[:, 0:CHUNKS[1]], AF.Exp, bias=b01[:],
                )
                # first xbar transpose as soon as the first 2048 cols exist
                nc.sync.dma_start_transpose(PT[:, 0:16, :], P[:, 0:2048])

                s2 = score_chunk(qsl, CHUNKS[0] + CHUNKS[1], CHUNKS[2])
                n2 = negmax(s2, CHUNKS[2], "n2")
                bias = stats.tile([128, 1], F32, tag="bias", name="bias")
                nc.vector.tensor_tensor(bias[:], b01[:], n2[:], op=OP.min)
                gin = stats.tile([128, 1], F32, tag="gin", name="gin")
                nc.vector.tensor_tensor(gin[:], bias[:], b01[:], op=OP.subtract)
                gam = stats.tile([128, 1], F32, tag="gam", name="gam")
                nc.scalar.activation(gam[:], gin[:], AF.Exp)
                nc.scalar.activation(
                    P[:, CHUNKS[0] + CHUNKS[1]:N],
                    s2[:, 0:CHUNKS[2]], AF.Exp, bias=bias[:],
                )
                nc.sync.dma_start_transpose(PT[:, 16:32, :], P[:, 2048:4096])
                cur = (PT, gam, i)
            else:
                cur = None

            if pend is not None:
                PTp, gamp, j = pend
                av = ps_av.tile([128, 129], F32, tag="av", name="av")
                for t in range(RESCALE_T):
                    nc.tensor.matmul(
                        av[:], PTp[:, t, :], vaug[:, t, :],
                        start=(t == 0), stop=False,
                    )
                # contributions so far were scaled with exp(-max01); bring to -max
                nc.scalar.activation(av[:], av[:], AF.Copy, bias=0.0, scale=gamp[:])
                for t in range(RESCALE_T, NT):
                    nc.tensor.matmul(
                        av[:], PTp[:, t, :], vaug[:, t, :],
                        start=False, stop=(t == NT - 1),
                    )
                linv = stats.tile([128, 1], F32, tag="linv", name="linv")
                nc.vector.reciprocal(linv[:], av[:, 128:129])
                ost = ostage.tile([128, 128], F32, tag="ost", name="ost")
                nc.scalar.activation(
                    ost[:], av[:, 0:128], AF.Copy, bias=0.0, scale=linv[:]
                )
                nc.sync.dma_start(out[j * 128:(j + 1) * 128, :], ost[:])
            pend = cur

    nc.compile()
    return nc


_NC_CACHE = {}


def _get_nc():
    if "nc" not in _NC_CACHE:
        nc = bacc.Bacc("TRN2", target_bir_lowering=False, debug=False, num_devices=B)
        _NC_CACHE["nc"] = build_attention(nc)
    return _NC_CACHE["nc"]


def kernel(x, w_query, w_key, w_value, _trace=False):
    x = np.ascontiguousarray(np.asarray(x, dtype=np.float32))
    w_query = np.ascontiguousarray(np.asarray(w_query, dtype=np.float32))
    w_key = np.ascontiguousarray(np.asarray(w_key, dtype=np.float32))
    w_value = np.ascontiguousarray(np.asarray(w_value, dtype=np.float32))
    nc = _get_nc()
    in_maps = [
        {"x": x[b], "w_query": w_query, "w_key": w_key, "w_value": w_value}
        for b in range(B)
    ]
    res = run_bass_kernel_spmd(nc, in_maps, core_ids=list(range(B)), trace=_trace)
    out_full = np.stack([res.results[b]["out"] for b in range(B)])
    if _trace:
        kernel.last_exec_time_ns = res.exec_time_ns
    return out_full
